# revision 13
# baseline (speedup 1.0000x reference)
"""Trainium2 Bass kernel for nn_AttentionPropagation.

Reference computation (per batch b):
  q = Wq@x1 + bq ; k = Wk@x2 + bk ; v = Wv@x2 + bv    (1x1 convs, [C, N])
  per head h (D=64): S = q_h^T k_h ; S = where(mask, S, -1e6)
  P = softmax(S / sqrt(D), axis=keys) ; attn = v_h @ P^T
  mh = Wmh@attn + bmh
  cat = [x1; mh] ; h = relu(BN(W1@cat + b1)) ; y = x1 + W2@h + b2

Sharding: 8 cores = (batch b in 0..3) x (query-half nh in 0..1).
Each core computes the full attention + MLP for its [C, 1024] query slice
against that batch's keys. Keys are compacted on the host (masked keys
dropped, padded to MPAD); padding columns get an exp bias of -125000 so
their softmax weight is exactly 0.

Attention layout (per core): scores are computed TRANSPOSED, S^T[m, n],
so the softmax denominator is folded into the AV matmul as an extra
all-ones column of v^T; no on-chip transposes are needed anywhere.

PE-array tiling: the score matmuls contract over D=64 only, so the two
heads of a pair (which live at SBUF partitions 0-63 / 64-127) are issued
back-to-back as independent row-group tiles (tile_position (0,0)/(64,0))
and execute CONCURRENTLY on the 128x128 PE array. The per-pair attention
outputs are stacked into one [128, NL] tile, which makes the mh matmul a
plain K=128 contraction (2 pair-chunks) at full array utilization.

Host-side folds (exact, float64):
  - BatchNorm folded into W1/b1.
  - bv folded downstream: normalized attn contributes +bv (sum of softmax
    weights = 1), so b1 += W1[:, C:] @ (Wmh@bv + bmh); kernel skips bv/bmh.
"""

import os
import sys

for _p in ("/opt/trn_rl_repo", "/root/.axon_site/_ro/trn_rl_repo"):
    if os.path.isdir(_p) and _p not in sys.path:
        sys.path.append(_p)

import ml_dtypes
import numpy as np

import concourse.bacc as bacc
import concourse.bass as bass
import concourse.mybir as mybir
import concourse.tile as tile
from concourse import bass_utils
from concourse.bass import ts

B, C, H, N, M = 4, 256, 4, 2048, 2048
D = C // H            # 64
NCORES = 8
NL = N // 2           # 1024 queries per core
MPAD = 1152           # padded (compacted) key count, multiple of 128
MC = MPAD // 128      # key chunks
BN_EPS = 1e-5
F32 = mybir.dt.float32
F32R = mybir.dt.float32r
BF16 = mybir.dt.bfloat16

# matmul-operand dtype: "bf16" (fast-weight-load + full PE rate) or "f32r"
# (tfloat32: ~5x lower error, slower weight loads)
MMDT_NAME = os.environ.get("KERNEL_MMDT", "bf16")
MMDT = {"bf16": BF16, "f32r": F32R}[MMDT_NAME]

# smalls layout: columns [bq(2) bk(2) b1(4) b2(2) maskb(MC)]
SM_BQ, SM_BK, SM_B1, SM_B2, SM_MB = 0, 2, 4, 8, 10
SM_W = SM_MB + MC


def build_nc():
    nc = bacc.Bacc("TRN2", target_bir_lowering=False, debug=False)

    dram = {}
    def din(name, shape, dt=F32):
        dram[name] = nc.dram_tensor(name, shape, dt, kind="ExternalInput").ap()
    din("x1r", [C, NL])
    din("x2c", [C, MPAD], MMDT)
    din("smalls", [128, SM_W])
    din("wqt", [C, C], MMDT)
    din("wkt", [C, C], MMDT)
    din("wvt", [C, C], MMDT)
    din("wmht", [C, C], MMDT)
    din("w1t", [2 * C, 2 * C], MMDT)
    din("w2t", [2 * C, C], MMDT)
    dram["y"] = nc.dram_tensor("y", [C, NL], F32, kind="ExternalOutput").ap()
    dram["dn"] = nc.dram_tensor("dn_bounce", [H, NL], F32).ap()

    with tile.TileContext(nc) as tc:
        build_kernel(tc, dram)
    nc.compile()
    return nc


def build_kernel(tc, dram):
    from contextlib import ExitStack
    nc = tc.nc
    ALU = mybir.AluOpType
    AF = mybir.ActivationFunctionType

    ctx = ExitStack()
    const = ctx.enter_context(tc.tile_pool(name="const", bufs=1))
    work = ctx.enter_context(tc.tile_pool(name="work", bufs=1))
    ptp = ctx.enter_context(tc.tile_pool(name="ptp", bufs=6))
    psum = ctx.enter_context(tc.tile_pool(name="psum", bufs=2, space="PSUM"))

    def mm(out, lhsT, rhs, start, stop):
        nc.tensor.matmul(out, lhsT, rhs, start=start, stop=stop)

    # ---- input/weight loads ----
    # each named load is one InstDMACopy (split across 16 SDMA engines);
    # descriptor-gen serializes per issuing engine, so spread across the
    # three DMA-capable queues by how soon that engine has compute to do.
    def load_tiles(eng, name, rows, width, nt, split=False):
        dt = dram[name].dtype
        big = const.tile([rows, nt, width], dt, tag=name, name=f"{name}_all")
        src = dram[name].rearrange("(b p) w -> p b w", p=rows)
        if split:
            for i in range(nt):
                eng[i % len(eng)].dma_start(out=big[:, i:i + 1, :],
                                            in_=src[:, i:i + 1, :])
        else:
            eng.dma_start(out=big, in_=src)
        return [big[:, i, :] for i in range(nt)]

    # sync: attention-phase DMAs come much later; front-load most loads here
    wkt_sb = load_tiles(nc.sync, "wkt", 128, C, 2)
    x2_sb = load_tiles([nc.gpsimd, nc.sync], "x2c", 128, MPAD, 2, split=True)
    smalls = const.tile([128, SM_W], F32, tag="smalls", name="smalls_sb")
    nc.gpsimd.dma_start(out=smalls, in_=dram["smalls"])
    x1r_sb = load_tiles([nc.scalar, nc.sync], "x1r", 128, NL, 2, split=True)
    wqt_sb = load_tiles(nc.scalar, "wqt", 128, C, 2)
    wvt_sb = load_tiles(nc.sync, "wvt", 128, C, 2)
    w1t_sb = load_tiles(nc.scalar, "w1t", 128, 2 * C, 4)
    wmht_sb = load_tiles(nc.sync, "wmht", 128, C, 2)
    w2t_sb = load_tiles(nc.scalar, "w2t", 128, C, 4)

    bq_s = smalls[:, SM_BQ:SM_BQ + 2]
    bk_s = smalls[:, SM_BK:SM_BK + 2]
    b1_s = smalls[:, SM_B1:SM_B1 + 4]
    b2_s = smalls[:, SM_B2:SM_B2 + 2]
    mb_s = smalls[:, SM_MB:SM_MB + MC]

    # ---- x1 cast to matmul dtype (q-proj + W1 rhs) ----
    x1c = []
    for cb in range(2):
        xc = work.tile([128, NL], MMDT, tag=f"x1c{cb}", name=f"x1c{cb}")
        nc.vector.tensor_copy(out=xc, in_=x1r_sb[cb])
        x1c.append(xc)

    # ---- k projection chunk emitter: k[cb] = [128 co, MPAD] ----
    k_sb = [work.tile([128, MPAD], MMDT, tag=f"k{cb}", name=f"k{cb}")
            for cb in range(2)]
    kchunks = [(0, 512), (512, 512), (1024, MPAD - 1024)]

    def emit_k_chunk(c):
        off, w = kchunks[c]
        for cb in range(2):
            ps = psum.tile([128, 512], F32, tag="st", name=f"k_ps{cb}_{off}")
            for kc in range(2):
                mm(ps[:, 0:w], wkt_sb[kc][:, ts(cb, 128)],
                   x2_sb[kc][:, off:off + w], start=(kc == 0), stop=(kc == 1))
            nc.vector.tensor_scalar_add(k_sb[cb][:, off:off + w], ps[:, 0:w],
                                        bk_s[:, cb:cb + 1])

    # ---- q projection: q[cb] = [128 co, NL] (before k: unblocks S sooner) ----
    q_sb = []
    for cb in range(2):
        ps = psum.tile([128, NL], F32, tag="st", name=f"q_ps{cb}")
        for kc in range(2):
            for nf in range(2):
                mm(ps[:, ts(nf, 512)], wqt_sb[kc][:, ts(cb, 128)],
                   x1c[kc][:, ts(nf, 512)], start=(kc == 0), stop=(kc == 1))
        qt = work.tile([128, NL], MMDT, tag=f"q{cb}", name=f"q{cb}")
        nc.vector.tensor_scalar_add(qt, ps, bq_s[:, cb:cb + 1])
        q_sb.append(qt)

    emit_k_chunk(0)

    # ---- v^T chunk emitter: vt[mc] = [128 m, 4*(D+1)] ----
    vt_sb = [None] * MC

    def emit_v_chunk(mc):
        ps = psum.tile([128, C], F32, tag="st", name=f"v_ps{mc}")
        for kc in range(2):
            mm(ps, x2_sb[kc][:, ts(mc, 128)], wvt_sb[kc],
               start=(kc == 0), stop=(kc == 1))
        vt = work.tile([128, H * (D + 1)], MMDT, tag=f"vt{mc}", name=f"vt{mc}")
        vt_r = vt.rearrange("p (h x) -> p h x", x=D + 1)
        nc.gpsimd.memset(vt_r[:, :, D:D + 1], 1.0)
        nc.vector.tensor_copy(out=vt_r[:, :, 0:D],
                              in_=ps.rearrange("p (h x) -> p h x", x=D))
        vt_sb[mc] = vt

    # ---- attention, one head-PAIR at a time ----
    # S^T pair: the two heads' k/q slices sit at partitions 0-63 / 64-127,
    # so their K=64 score matmuls land on disjoint PE row groups (auto
    # tile_position (0,0) / (64,0)) and run concurrently.
    attn = []   # attn[p] = [128, NL] pair-stacked normalized attention
    for p in range(2):
        av = [psum.tile([D + 1, NL], F32, tag="av", name=f"av{p}_{hh}")
              for hh in range(2)]
        pts = [None, None]
        for mc in range(MC):
            sts = [psum.tile([128, NL], F32, tag="st", name=f"st{p}_{hh}_{mc}")
                   for hh in range(2)]
            for nf in range(2):
                for hh in range(2):
                    off = hh * D
                    mm(sts[hh][:, ts(nf, 512)],
                       k_sb[p][off:off + D, ts(mc, 128)],
                       q_sb[p][off:off + D, ts(nf, 512)],
                       start=True, stop=True)
            if p == 0:
                # JIT v^T / k production: these PE slots share the stall
                # in which AV waits for this chunk's exp anyway.
                emit_v_chunk(mc)
                if mc in (3, 7):
                    emit_k_chunk(mc // 4 + 1)
            for hh in range(2):
                pt = ptp.tile([128, NL], MMDT, tag="pt", name=f"pt{p}_{hh}_{mc}")
                nc.scalar.activation(out=pt, in_=sts[hh], func=AF.Exp,
                                     bias=mb_s[:, mc:mc + 1], scale=0.125)
                pts[hh] = pt
            for hh in range(2):
                h = 2 * p + hh
                for nf in range(2):
                    mm(av[hh][:, ts(nf, 512)],
                       vt_sb[mc][:, h * (D + 1):(h + 1) * (D + 1)],
                       pts[hh][:, ts(nf, 512)],
                       start=(mc == 0), stop=(mc == MC - 1))

        # copy raw attention out of PSUM immediately (frees the av bank
        # pair for the next accumulation while the denominator chain runs)
        araw = [work.tile([D + 1, NL], F32, tag="araw", name=f"araw{p}_{hh}",
                          bufs=2)
                for hh in range(2)]
        for hh in range(2):
            nc.vector.tensor_copy(out=araw[hh], in_=av[hh])

        # denominator: DMA-scatter the two denominator rows across 64
        # partitions x 32 lanes, exact reciprocal there (~0.2us), bounce
        # through DRAM for the partition-broadcast read of both heads.
        den = work.tile([64, 32], F32, tag="den", name=f"den{p}")
        den_r = den.rearrange("p (e j) -> p e j", j=16)
        for hh in range(2):
            nc.sync.dma_start(out=den_r[:, hh, :], in_=araw[hh][D:D + 1, :])
        rcp = work.tile([64, 32], F32, tag="rcp", name=f"rcp{p}")
        nc.vector.reciprocal(out=rcp, in_=den)
        dnt = dram["dn"]
        dn_scat = bass.AP(tensor=dnt.tensor, offset=2 * p * NL,
                          ap=[[16, 64], [NL, 2], [1, 16]])
        nc.sync.dma_start(out=dn_scat,
                          in_=rcp.rearrange("p (e j) -> p e j", j=16))
        # normalize both heads at base partition 0 (DVE lanes are
        # partition-locked), then DMA-stack the odd head into rows 64-127
        # so mh can contract over the pair at K=128. Split per query-half
        # so downstream matmuls start on nf0 while nf1 still normalizes.
        bc = work.tile([D, 2, NL], F32, tag="bc", name=f"bc{p}")
        at = work.tile([128, NL], MMDT, tag=f"attn{p}", name=f"attn{p}")
        at1 = work.tile([D, NL], MMDT, tag="at1", name=f"at1_{p}")
        for nf in range(2):
            sl = ts(nf, 512)
            bcast_ap = bass.AP(tensor=dnt.tensor, offset=2 * p * NL + nf * 512,
                               ap=[[0, D], [NL, 2], [1, 512]])
            nc.sync.dma_start(out=bc[:, :, sl], in_=bcast_ap)
            nc.vector.tensor_mul(out=at[0:D, sl], in0=araw[0][0:D, sl],
                                 in1=bc[:, 0, sl])
            nc.vector.tensor_mul(out=at1[:, sl], in0=araw[1][0:D, sl],
                                 in1=bc[:, 1, sl])
            nc.sync.dma_start(out=at[D:2 * D, sl], in_=at1[:, sl])
        attn.append(at)

    # ---- h1a = W1[:, :C] @ x1 partial (independent of attention; fills
    #      the PE while the pair-1 denominator chain drains) ----
    h1a_sb = []
    for ob in range(4):
        ps = psum.tile([128, NL], F32, tag="st", name=f"h1a_ps{ob}")
        for kc in range(2):
            for nf in range(2):
                mm(ps[:, ts(nf, 512)], w1t_sb[kc][:, ts(ob, 128)],
                   x1c[kc][:, ts(nf, 512)], start=(kc == 0), stop=(kc == 1))
        ht = work.tile([128, NL], F32, tag="h1a", name=f"h1a{ob}", bufs=4)
        nc.vector.tensor_copy(out=ht, in_=ps)
        h1a_sb.append(ht)

    # ---- mh = Wmh^T.T @ attn (pair-stacked: K=128 per chunk) ----
    mh_sb = []
    for cb in range(2):
        ps = psum.tile([128, NL], F32, tag="st", name=f"mh_ps{cb}")
        for pp in range(2):
            for nf in range(2):
                mm(ps[:, ts(nf, 512)], wmht_sb[pp][:, ts(cb, 128)],
                   attn[pp][:, ts(nf, 512)], start=(pp == 0), stop=(pp == 1))
        mt = work.tile([128, NL], MMDT, tag=f"mh{cb}", name=f"mh{cb}")
        nc.vector.tensor_copy(out=mt, in_=ps)
        mh_sb.append(mt)

    # ---- h1 = relu(h1a + W1[:, C:] @ mh + b1); y = x1 + W2 @ h1 + b2 ----
    yps = [psum.tile([128, NL], F32, tag="av", name=f"y_ps{cb}")
           for cb in range(2)]
    for ob in range(4):
        ps = psum.tile([128, NL], F32, tag="st", name=f"h1b_ps{ob}")
        for kc in range(2):
            for nf in range(2):
                mm(ps[:, ts(nf, 512)], w1t_sb[2 + kc][:, ts(ob, 128)],
                   mh_sb[kc][:, ts(nf, 512)], start=(kc == 0), stop=(kc == 1))
        hsum = work.tile([128, NL], F32, tag="hsum", name=f"hsum{ob}", bufs=2)
        nc.vector.tensor_add(out=hsum, in0=ps, in1=h1a_sb[ob])
        ht = work.tile([128, NL], MMDT, tag="h1", name=f"h1{ob}", bufs=2)
        nc.scalar.activation(out=ht, in_=hsum, func=AF.Relu,
                             bias=b1_s[:, ob:ob + 1])
        for cb in range(2):
            for nf in range(2):
                mm(yps[cb][:, ts(nf, 512)], w2t_sb[ob][:, ts(cb, 128)],
                   ht[:, ts(nf, 512)], start=(ob == 0), stop=(ob == 3))
    for cb in range(2):
        yt = work.tile([128, NL], F32, tag=f"y{cb}", name=f"y{cb}")
        nc.vector.scalar_tensor_tensor(out=yt, in0=yps[cb],
                                       scalar=b2_s[:, cb:cb + 1],
                                       in1=x1r_sb[cb],
                                       op0=ALU.add, op1=ALU.add)
        nc.sync.dma_start(out=dram["y"][ts(cb, 128), :], in_=yt)

    ctx.close()


# ---------------------------------------------------------------------------
# host side
# ---------------------------------------------------------------------------

_NC_CACHE = {}


def _get_nc():
    if "nc" not in _NC_CACHE:
        _NC_CACHE["nc"] = build_nc()
    return _NC_CACHE["nc"]


def kernel(x1, x2, kv_mask, Wq, bq, Wk, bk, Wv, bv, Wmh, bmh,
           W1, b1, bn_gamma, bn_beta, bn_mean, bn_var, W2, b2):
    x1 = np.asarray(x1, np.float32)
    x2 = np.asarray(x2, np.float32)
    kv_mask = np.asarray(kv_mask).astype(bool)
    Wq, Wk, Wv, Wmh = (np.asarray(a, np.float32) for a in (Wq, Wk, Wv, Wmh))
    W1, W2 = np.asarray(W1, np.float32), np.asarray(W2, np.float32)
    bqv, bkv, bvv, bmhv = (np.asarray(a, np.float64) for a in (bq, bk, bv, bmh))
    b1v, b2v = np.asarray(b1, np.float64), np.asarray(b2, np.float64)
    g, bt = np.asarray(bn_gamma, np.float64), np.asarray(bn_beta, np.float64)
    mu, var = np.asarray(bn_mean, np.float64), np.asarray(bn_var, np.float64)

    # fold BN into W1/b1; fold bv/bmh into b1 (exact, float64)
    s = g / np.sqrt(var + BN_EPS)
    W1f = s[:, None] * W1.astype(np.float64)
    b1f = s * (b1v - mu) + bt
    b1f = b1f + W1f[:, C:] @ (np.asarray(Wmh, np.float64) @ bvv + bmhv)
    W1f32 = W1f.astype(np.float32)

    mmnp = {"bf16": ml_dtypes.bfloat16, "f32r": np.float32}[MMDT_NAME]
    shared = {
        "wqt": np.ascontiguousarray(Wq.T).astype(mmnp),
        "wkt": np.ascontiguousarray(Wk.T).astype(mmnp),
        "wvt": np.ascontiguousarray(Wv.T).astype(mmnp),
        "wmht": np.ascontiguousarray(Wmh.T).astype(mmnp),
        "w1t": np.ascontiguousarray(W1f32.T).astype(mmnp),
        "w2t": np.ascontiguousarray(W2.T).astype(mmnp),
    }

    in_maps = []
    for core in range(NCORES):
        b, nh = core // 2, core % 2
        idx = np.nonzero(kv_mask[b])[0]
        mb = len(idx)
        assert mb <= MPAD, f"batch {b}: {mb} unmasked keys > MPAD={MPAD}"
        x2c = np.zeros((C, MPAD), np.float32)
        x2c[:, :mb] = x2[b][:, idx]
        mbias = np.full(MPAD, -125000.0, np.float32)
        mbias[:mb] = 0.0
        smalls = np.zeros((128, SM_W), np.float32)
        smalls[:, SM_BQ:SM_BQ + 2] = bqv.astype(np.float32).reshape(2, 128).T
        smalls[:, SM_BK:SM_BK + 2] = bkv.astype(np.float32).reshape(2, 128).T
        smalls[:, SM_B1:SM_B1 + 4] = b1f.astype(np.float32).reshape(4, 128).T
        smalls[:, SM_B2:SM_B2 + 2] = b2v.astype(np.float32).reshape(2, 128).T
        smalls[:, SM_MB:SM_MB + MC] = mbias.reshape(MC, 128).T
        im = dict(shared)
        im["x1r"] = np.ascontiguousarray(x1[b][:, nh * NL:(nh + 1) * NL])
        im["x2c"] = x2c.astype(mmnp)
        im["smalls"] = smalls
        in_maps.append(im)

    nc = _get_nc()
    res = bass_utils.run_bass_kernel_spmd(nc, in_maps, core_ids=list(range(NCORES)))
    _NC_CACHE["last_res"] = res

    out = np.empty((B, C, N), np.float32)
    for core in range(NCORES):
        b, nh = core // 2, core % 2
        out[b][:, nh * NL:(nh + 1) * NL] = res.results[core]["y"]
    return out


if __name__ == "__main__":
    build_nc()
    print("built + compiled OK")


# revision 24
# speedup vs baseline: 1.2613x; 1.2613x over previous
"""Trainium2 Bass kernel for nn_AttentionPropagation.

Reference computation (per batch b):
  q = Wq@x1 + bq ; k = Wk@x2 + bk ; v = Wv@x2 + bv    (1x1 convs, [C, N])
  per head h (D=64): S = q_h^T k_h ; S = where(mask, S, -1e6)
  P = softmax(S / sqrt(D), axis=keys) ; attn = v_h @ P^T
  mh = Wmh@attn + bmh
  cat = [x1; mh] ; h = relu(BN(W1@cat + b1)) ; y = x1 + W2@h + b2

Sharding: 8 cores = (batch b in 0..3) x (query-half nh in 0..1).
Each core computes the full attention + MLP for its [C, 1024] query slice
against that batch's keys. Keys are compacted on the host (masked keys
dropped, padded to MPAD); padding columns get an exp bias of -125000 so
their softmax weight is exactly 0.

Attention layout (per core): scores are computed TRANSPOSED, S^T[m, n],
so the softmax denominator is folded into the AV matmul as an extra
all-ones column of v^T; no on-chip transposes are needed anywhere.

PE-array tiling: the score matmuls contract over D=64 only, so the two
heads of a pair (which live at SBUF partitions 0-63 / 64-127) are issued
back-to-back as independent row-group tiles (tile_position (0,0)/(64,0))
and execute CONCURRENTLY on the 128x128 PE array. The per-pair attention
outputs are stacked into one [128, NL] tile, which makes the mh matmul a
plain K=128 contraction (2 pair-chunks) at full array utilization.

Host-side folds (exact, float64):
  - BatchNorm folded into W1/b1.
  - bv folded downstream: normalized attn contributes +bv (sum of softmax
    weights = 1), so b1 += W1[:, C:] @ (Wmh@bv + bmh); kernel skips bv/bmh.
"""

import os
import sys

for _p in ("/opt/trn_rl_repo", "/root/.axon_site/_ro/trn_rl_repo"):
    if os.path.isdir(_p) and _p not in sys.path:
        sys.path.append(_p)

import ml_dtypes
import numpy as np

import concourse.bacc as bacc
import concourse.bass as bass
import concourse.mybir as mybir
import concourse.tile as tile
from concourse import bass_utils
from concourse.bass import ts

B, C, H, N, M = 4, 256, 4, 2048, 2048
D = C // H            # 64
NCORES = 8
NL = N // 2           # 1024 queries per core
MPAD = 1152           # padded (compacted) key count, multiple of 128
MC = MPAD // 128      # key chunks
BN_EPS = 1e-5
F32 = mybir.dt.float32
F32R = mybir.dt.float32r
BF16 = mybir.dt.bfloat16

# matmul-operand dtype: "bf16" (fast-weight-load + full PE rate) or "f32r"
# (tfloat32: ~5x lower error, slower weight loads)
MMDT_NAME = os.environ.get("KERNEL_MMDT", "bf16")
MMDT = {"bf16": BF16, "f32r": F32R}[MMDT_NAME]

# smalls layout: columns [bq(2) bk(2) b1(4) b2(2) maskb(MC)]
SM_BQ, SM_BK, SM_B1, SM_B2, SM_MB = 0, 2, 4, 8, 10
SM_W = SM_MB + MC


def build_nc():
    nc = bacc.Bacc("TRN2", target_bir_lowering=False, debug=False)

    dram = {}
    def din(name, shape, dt=F32):
        dram[name] = nc.dram_tensor(name, shape, dt, kind="ExternalInput").ap()
    din("x1r", [C, NL])
    din("x2c", [C, MPAD], MMDT)
    din("smalls", [128, SM_W])
    din("wqt", [C, C], MMDT)
    din("wkt", [C, C], MMDT)
    din("wvt", [C, C], MMDT)
    din("wmht", [C, C], MMDT)
    din("w1t", [2 * C, 2 * C], MMDT)
    din("w2t", [2 * C, C], MMDT)
    dram["y"] = nc.dram_tensor("y", [C, NL], F32, kind="ExternalOutput").ap()
    dram["dn"] = nc.dram_tensor("dn_bounce", [H, NL], F32).ap()

    with tile.TileContext(nc) as tc:
        build_kernel(tc, dram)
    nc.compile()
    return nc


def build_kernel(tc, dram):
    from contextlib import ExitStack
    nc = tc.nc
    ALU = mybir.AluOpType
    AF = mybir.ActivationFunctionType

    ctx = ExitStack()
    const = ctx.enter_context(tc.tile_pool(name="const", bufs=1))
    work = ctx.enter_context(tc.tile_pool(name="work", bufs=1))
    ptp = ctx.enter_context(tc.tile_pool(name="ptp", bufs=6))
    psum = ctx.enter_context(tc.tile_pool(name="psum", bufs=2, space="PSUM"))

    def mm(out, lhsT, rhs, start, stop, tile_position=None):
        nc.tensor.matmul(out, lhsT, rhs, start=start, stop=stop,
                         tile_position=tile_position)

    # ---- input/weight loads ----
    # each named load is one InstDMACopy (split across 16 SDMA engines);
    # descriptor-gen serializes per issuing engine, so spread across the
    # three DMA-capable queues by how soon that engine has compute to do.
    def load_tiles(eng, name, rows, width, nt, split=False):
        dt = dram[name].dtype
        big = const.tile([rows, nt, width], dt, tag=name, name=f"{name}_all")
        src = dram[name].rearrange("(b p) w -> p b w", p=rows)
        if split:
            for i in range(nt):
                eng[i % len(eng)].dma_start(out=big[:, i:i + 1, :],
                                            in_=src[:, i:i + 1, :])
        else:
            eng.dma_start(out=big, in_=src)
        return [big[:, i, :] for i in range(nt)]

    # sync: attention-phase DMAs come much later; front-load most loads here
    wkt_sb = load_tiles(nc.sync, "wkt", 128, C, 2)
    x2_sb = load_tiles([nc.gpsimd, nc.sync], "x2c", 128, MPAD, 2, split=True)
    smalls = const.tile([128, SM_W], F32, tag="smalls", name="smalls_sb")
    nc.gpsimd.dma_start(out=smalls, in_=dram["smalls"])
    x1r_sb = load_tiles([nc.scalar, nc.sync], "x1r", 128, NL, 2, split=True)
    wqt_sb = load_tiles(nc.scalar, "wqt", 128, C, 2)
    wvt_sb = load_tiles(nc.sync, "wvt", 128, C, 2)
    w1t_sb = load_tiles(nc.scalar, "w1t", 128, 2 * C, 4)
    wmht_sb = load_tiles(nc.sync, "wmht", 128, C, 2)
    w2t_sb = load_tiles(nc.scalar, "w2t", 128, C, 4)

    bq_s = smalls[:, SM_BQ:SM_BQ + 2]
    bk_s = smalls[:, SM_BK:SM_BK + 2]
    b1_s = smalls[:, SM_B1:SM_B1 + 4]
    b2_s = smalls[:, SM_B2:SM_B2 + 2]
    mb_s = smalls[:, SM_MB:SM_MB + MC]

    # ---- x1 cast to matmul dtype (q-proj + W1 rhs) ----
    x1c = []
    for cb in range(2):
        xc = work.tile([128, NL], MMDT, tag=f"x1c{cb}", name=f"x1c{cb}")
        nc.vector.tensor_copy(out=xc, in_=x1r_sb[cb])
        x1c.append(xc)

    # ---- k projection chunk emitter: k[cb] = [128 co, MPAD] ----
    k_sb = [work.tile([128, MPAD], MMDT, tag=f"k{cb}", name=f"k{cb}")
            for cb in range(2)]
    kchunks = [(0, 512), (512, 512), (1024, MPAD - 1024)]

    def emit_k_chunk(c):
        off, w = kchunks[c]
        for cb in range(2):
            ps = psum.tile([128, 512], F32, tag="st", name=f"k_ps{cb}_{off}")
            for kc in range(2):
                mm(ps[:, 0:w], wkt_sb[kc][:, ts(cb, 128)],
                   x2_sb[kc][:, off:off + w], start=(kc == 0), stop=(kc == 1))
            nc.vector.tensor_scalar_add(k_sb[cb][:, off:off + w], ps[:, 0:w],
                                        bk_s[:, cb:cb + 1])

    # ---- q projection: q[cb] = [128 co, NL] (before k: unblocks S sooner) ----
    q_sb = []
    for cb in range(2):
        ps = psum.tile([128, NL], F32, tag="st", name=f"q_ps{cb}")
        for kc in range(2):
            for nf in range(2):
                mm(ps[:, ts(nf, 512)], wqt_sb[kc][:, ts(cb, 128)],
                   x1c[kc][:, ts(nf, 512)], start=(kc == 0), stop=(kc == 1))
        qt = work.tile([128, NL], MMDT, tag=f"q{cb}", name=f"q{cb}")
        nc.vector.tensor_scalar_add(qt, ps, bq_s[:, cb:cb + 1])
        q_sb.append(qt)

    emit_k_chunk(0)

    # ---- v^T chunk emitter: vt[mc] = [128 m, H*D] ----
    vt_sb = [None] * MC

    def emit_v_chunk(mc):
        ps = psum.tile([128, C], F32, tag="st", name=f"v_ps{mc}")
        for kc in range(2):
            mm(ps, x2_sb[kc][:, ts(mc, 128)], wvt_sb[kc],
               start=(kc == 0), stop=(kc == 1))
        vt = work.tile([128, H * D], MMDT, tag=f"vt{mc}", name=f"vt{mc}")
        nc.vector.tensor_copy(out=vt, in_=ps)
        vt_sb[mc] = vt

    ones_sb = work.tile([128, 32], MMDT, tag="ones", name="ones_sb")
    nc.gpsimd.memset(ones_sb, 1.0)

    # ---- h1a = W1[:, :C] @ x1 partial emitter (PE filler work) ----
    h1a_sb = [None] * 4

    def emit_h1a(ob):
        ps = psum.tile([128, NL], F32, tag="st", name=f"h1a_ps{ob}")
        for kc in range(2):
            for nf in range(2):
                mm(ps[:, ts(nf, 512)], w1t_sb[kc][:, ts(ob, 128)],
                   x1c[kc][:, ts(nf, 512)], start=(kc == 0), stop=(kc == 1))
        ht = work.tile([128, NL], F32, tag="h1a", name=f"h1a{ob}", bufs=4)
        nc.vector.tensor_copy(out=ht, in_=ps)
        h1a_sb[ob] = ht

    # ---- attention, one head-PAIR at a time ----
    # S^T pair: the two heads' k/q slices sit at partitions 0-63 / 64-127,
    # so their K=64 score matmuls land on disjoint PE row groups (auto
    # tile_position (0,0) / (64,0)) and run concurrently. The AV matmuls
    # are col-group-paired (M=64 tiles at col 0 / col 64) writing one
    # pair-stacked [128, NL] accumulator - also concurrent. Denominators
    # come from M=1 ones-matmuls into a shared [97, NL] tile, emitted one
    # chunk late so they fill the PE slot where AV waits for this chunk's
    # exp. PSUM: st 4 banks + av 2 + den 2 = 8.
    den_ps = psum.tile([128, NL], F32, tag="den_ps", name="den_ps", bufs=1)

    def emit_den(p, mc, pts_prev):
        for hh in range(2):
            r = 32 * (2 * p + hh)
            for nf in range(2):
                mm(den_ps[r:r + 32, ts(nf, 512)], ones_sb,
                   pts_prev[hh][:, ts(nf, 512)],
                   start=(mc == 0), stop=(mc == MC - 1),
                   tile_position=(0, r))

    attn = []   # attn[p] = [128, NL] pair-stacked normalized attention
    dnt = dram["dn"]
    for p in range(2):
        av = psum.tile([128, NL], F32, tag="av", name=f"av{p}", bufs=1)
        pts_prev = None
        for mc in range(MC):
            sts = [psum.tile([128, NL], F32, tag="st", name=f"st{p}_{hh}_{mc}")
                   for hh in range(2)]
            for nf in range(2):
                for hh in range(2):
                    off = hh * D
                    mm(sts[hh][:, ts(nf, 512)],
                       k_sb[p][off:off + D, ts(mc, 128)],
                       q_sb[p][off:off + D, ts(nf, 512)],
                       start=True, stop=True)
            # filler PE work that does not depend on this chunk's exp:
            # previous chunk's denominator matmuls + JIT v^T / k chunks
            if pts_prev is not None:
                emit_den(p, mc - 1, pts_prev)
            if p == 0:
                emit_v_chunk(mc)
                if mc in (3, 7):
                    emit_k_chunk(mc // 4 + 1)
            pts = [None, None]
            for hh in range(2):
                pt = ptp.tile([128, NL], MMDT, tag="pt", name=f"pt{p}_{hh}_{mc}")
                nc.scalar.activation(out=pt, in_=sts[hh], func=AF.Exp,
                                     bias=mb_s[:, mc:mc + 1], scale=0.125)
                pts[hh] = pt
            for hh in range(2):
                h = 2 * p + hh
                for nf in range(2):
                    mm(av[ts(hh, D), ts(nf, 512)],
                       vt_sb[mc][:, h * D:(h + 1) * D],
                       pts[hh][:, ts(nf, 512)],
                       start=(mc == 0), stop=(mc == MC - 1))
            pts_prev = pts
        emit_den(p, MC - 1, pts_prev)

        # copy raw pair-stacked attention out of PSUM immediately (frees
        # the av bank pair for the next pair while the chain runs)
        araw = work.tile([128, NL], F32, tag="araw", name=f"araw{p}", bufs=2)
        nc.vector.tensor_copy(out=araw, in_=av)

        # denominator chain: copy this pair's den rows to SBUF, DMA-scatter
        # each [1, NL] row across 64 partitions x 16 lanes, exact reciprocal
        # there, bounce through DRAM for the pair-stacked partition-broadcast
        # read, then normalize into the attn tile per query-half.
        dsb = work.tile([97, NL], F32, tag="dsb", name=f"dsb{p}", bufs=2)
        r0 = 64 * p
        nc.vector.tensor_copy(out=dsb[r0:r0 + 33, :],
                              in_=den_ps[r0:r0 + 33, :])
        den = work.tile([64, 32], F32, tag="den", name=f"den{p}")
        den_r = den.rearrange("p (e j) -> p e j", j=16)
        for hh in range(2):
            nc.sync.dma_start(out=den_r[:, hh, :],
                              in_=dsb[r0 + 32 * hh:r0 + 32 * hh + 1, :])
        rcp = work.tile([64, 32], F32, tag="rcp", name=f"rcp{p}")
        nc.vector.reciprocal(out=rcp, in_=den)
        dn_scat = bass.AP(tensor=dnt.tensor, offset=2 * p * NL,
                          ap=[[16, 64], [NL, 2], [1, 16]])
        nc.sync.dma_start(out=dn_scat,
                          in_=rcp.rearrange("p (e j) -> p e j", j=16))
        bc = work.tile([128, NL], F32, tag="bc", name=f"bc{p}")
        at = work.tile([128, NL], MMDT, tag=f"attn{p}", name=f"attn{p}")
        for hh in range(2):
            row = dnt[2 * p + hh:2 * p + hh + 1, :]
            bcast_ap = bass.AP(tensor=row.tensor, offset=row.offset,
                               ap=[[0, D]] + list(row.ap[1:]))
            nc.sync.dma_start(out=bc[ts(hh, D), :], in_=bcast_ap)
        for nf in range(2):
            sl = ts(nf, 512)
            nc.vector.tensor_mul(out=at[:, sl], in0=araw[:, sl],
                                 in1=bc[:, sl])
        attn.append(at)

    # W1-x1 partials: PE filler covering the pair-1 denominator chain
    for ob in range(4):
        emit_h1a(ob)

    # ---- mh = Wmh^T.T @ attn (pair-stacked: K=128 per chunk) ----
    mh_sb = []
    for cb in range(2):
        ps = psum.tile([128, NL], F32, tag="st", name=f"mh_ps{cb}")
        for pp in range(2):
            for nf in range(2):
                mm(ps[:, ts(nf, 512)], wmht_sb[pp][:, ts(cb, 128)],
                   attn[pp][:, ts(nf, 512)], start=(pp == 0), stop=(pp == 1))
        mt = work.tile([128, NL], MMDT, tag=f"mh{cb}", name=f"mh{cb}")
        nc.vector.tensor_copy(out=mt, in_=ps)
        mh_sb.append(mt)

    # ---- h1 = relu(h1a + W1[:, C:] @ mh + b1); y = x1 + W2 @ h1 + b2 ----
    yps = [psum.tile([128, NL], F32, tag="av", name="y_ps0", bufs=1),
           psum.tile([128, NL], F32, tag="den_ps", name="y_ps1", bufs=1)]
    for ob in range(4):
        ps = psum.tile([128, NL], F32, tag="st", name=f"h1b_ps{ob}")
        for kc in range(2):
            for nf in range(2):
                mm(ps[:, ts(nf, 512)], w1t_sb[2 + kc][:, ts(ob, 128)],
                   mh_sb[kc][:, ts(nf, 512)], start=(kc == 0), stop=(kc == 1))
        hsum = work.tile([128, NL], F32, tag="hsum", name=f"hsum{ob}", bufs=2)
        nc.vector.tensor_add(out=hsum, in0=ps, in1=h1a_sb[ob])
        ht = work.tile([128, NL], MMDT, tag="h1", name=f"h1{ob}", bufs=2)
        nc.scalar.activation(out=ht, in_=hsum, func=AF.Relu,
                             bias=b1_s[:, ob:ob + 1])
        for cb in range(2):
            for nf in range(2):
                mm(yps[cb][:, ts(nf, 512)], w2t_sb[ob][:, ts(cb, 128)],
                   ht[:, ts(nf, 512)], start=(ob == 0), stop=(ob == 3))
    for cb in range(2):
        yt = work.tile([128, NL], F32, tag=f"y{cb}", name=f"y{cb}")
        nc.vector.scalar_tensor_tensor(out=yt, in0=yps[cb],
                                       scalar=b2_s[:, cb:cb + 1],
                                       in1=x1r_sb[cb],
                                       op0=ALU.add, op1=ALU.add)
        nc.sync.dma_start(out=dram["y"][ts(cb, 128), :], in_=yt)

    ctx.close()


# ---------------------------------------------------------------------------
# host side
# ---------------------------------------------------------------------------

_NC_CACHE = {}


def _get_nc():
    if "nc" not in _NC_CACHE:
        _NC_CACHE["nc"] = build_nc()
    return _NC_CACHE["nc"]


def kernel(x1, x2, kv_mask, Wq, bq, Wk, bk, Wv, bv, Wmh, bmh,
           W1, b1, bn_gamma, bn_beta, bn_mean, bn_var, W2, b2):
    x1 = np.asarray(x1, np.float32)
    x2 = np.asarray(x2, np.float32)
    kv_mask = np.asarray(kv_mask).astype(bool)
    Wq, Wk, Wv, Wmh = (np.asarray(a, np.float32) for a in (Wq, Wk, Wv, Wmh))
    W1, W2 = np.asarray(W1, np.float32), np.asarray(W2, np.float32)
    bqv, bkv, bvv, bmhv = (np.asarray(a, np.float64) for a in (bq, bk, bv, bmh))
    b1v, b2v = np.asarray(b1, np.float64), np.asarray(b2, np.float64)
    g, bt = np.asarray(bn_gamma, np.float64), np.asarray(bn_beta, np.float64)
    mu, var = np.asarray(bn_mean, np.float64), np.asarray(bn_var, np.float64)

    # fold BN into W1/b1; fold bv/bmh into b1 (exact, float64)
    s = g / np.sqrt(var + BN_EPS)
    W1f = s[:, None] * W1.astype(np.float64)
    b1f = s * (b1v - mu) + bt
    b1f = b1f + W1f[:, C:] @ (np.asarray(Wmh, np.float64) @ bvv + bmhv)
    W1f32 = W1f.astype(np.float32)

    mmnp = {"bf16": ml_dtypes.bfloat16, "f32r": np.float32}[MMDT_NAME]
    shared = {
        "wqt": np.ascontiguousarray(Wq.T).astype(mmnp),
        "wkt": np.ascontiguousarray(Wk.T).astype(mmnp),
        "wvt": np.ascontiguousarray(Wv.T).astype(mmnp),
        "wmht": np.ascontiguousarray(Wmh.T).astype(mmnp),
        "w1t": np.ascontiguousarray(W1f32.T).astype(mmnp),
        "w2t": np.ascontiguousarray(W2.T).astype(mmnp),
    }

    in_maps = []
    for core in range(NCORES):
        b, nh = core // 2, core % 2
        idx = np.nonzero(kv_mask[b])[0]
        mb = len(idx)
        assert mb <= MPAD, f"batch {b}: {mb} unmasked keys > MPAD={MPAD}"
        x2c = np.zeros((C, MPAD), np.float32)
        x2c[:, :mb] = x2[b][:, idx]
        mbias = np.full(MPAD, -125000.0, np.float32)
        mbias[:mb] = 0.0
        smalls = np.zeros((128, SM_W), np.float32)
        smalls[:, SM_BQ:SM_BQ + 2] = bqv.astype(np.float32).reshape(2, 128).T
        smalls[:, SM_BK:SM_BK + 2] = bkv.astype(np.float32).reshape(2, 128).T
        smalls[:, SM_B1:SM_B1 + 4] = b1f.astype(np.float32).reshape(4, 128).T
        smalls[:, SM_B2:SM_B2 + 2] = b2v.astype(np.float32).reshape(2, 128).T
        smalls[:, SM_MB:SM_MB + MC] = mbias.reshape(MC, 128).T
        im = dict(shared)
        im["x1r"] = np.ascontiguousarray(x1[b][:, nh * NL:(nh + 1) * NL])
        im["x2c"] = x2c.astype(mmnp)
        im["smalls"] = smalls
        in_maps.append(im)

    nc = _get_nc()
    res = bass_utils.run_bass_kernel_spmd(nc, in_maps, core_ids=list(range(NCORES)))
    _NC_CACHE["last_res"] = res

    out = np.empty((B, C, N), np.float32)
    for core in range(NCORES):
        b, nh = core // 2, core % 2
        out[b][:, nh * NL:(nh + 1) * NL] = res.results[core]["y"]
    return out


if __name__ == "__main__":
    build_nc()
    print("built + compiled OK")
